# revision 80
# baseline (speedup 1.0000x reference)
"""Attention-Augmented Conv2D (AAConv2D) distributed Bass kernel for 8 TRN2 NeuronCores.

Strategy: pure data-parallel over batch (B=8 -> one image per core, weights
replicated, zero collectives). Per core, for one [32,32,256] image:

  conv branch : 3x3 SAME conv (256->256ch) as 9 shifted-window matmuls
                accumulated in PSUM, one tap per attention chunk-slot
                (PE filler under the ACT-bound softmax stream).
  attn branch : kqv 1x1 conv (channel-major k/q, position-major v),
                per-head S^T = K Q^T computed with an AUGMENTED contraction
                (32 qk dims + 64 one-hot w/h-offset dims = 96) so the
                relative-position logits ride in the same matmul stream;
                exp on ScalarE (max-free softmax, logits are O(10) so fp32
                exp is safe); P^T V via matmul with [v|1] stationary
                (sumexp ride-along); per-head normalize on VectorE/GpSimd;
                output projection at the tail.

Schedule notes (v3 — rebuilt from the v2 trace):
  - v2 spent 53us before the first exp and had ~4us inter-head ACT
    bubbles (conv+rel matmuls queued between heads) plus 92us of HAM
    half-clock. v3 pipelines one global chunk stream: per chunk slot the
    PE does [S(next), PV(prev), 1 conv tap] (~1.05us) under the 1.11us
    exp, with no inter-head break in the rotation.
  - All rel-logit matmuls run upfront in two 4-head-concurrent waves,
    reading q directly from the kqv-escape strips (krw4/krh4 tables are
    strip-replicated host-side) — the v2 qrep replication DMAs are gone.
  - The one-hot delta rows live in two parity stationary tiles st0/st1
    (rows 32:96 written once from DRAM); only the 32 k-rows are
    re-DMA'd per head, two heads ahead, on the sync queue.
  - The scalar queue carries NO DMAs after startup (v2 put 26us of
    descriptor time on it, starving exp issue); exp's ACT table is
    preloaded by a dummy activation at t~0.
  - Input DMAs fan across sync/vector/scalar/gpsimd queues in need
    order (xtc+wkqv first).
  - PSUM (8 banks): S rotation 2x[128,1024]f32 (4) + PV accumulator
    [128,1024]f32 (2) + conv/startup/misc tile (2).

All heavy matmuls in bf16 (fp32 matmul is 4x slower on TRN2 PE).
Host does layout-only prep: batch sharding, transposes to channel-major,
bf16 casts, relative-table window expansion, one-hot delta tables, and the
exact algebraic fold of the v-bias into the projection bias.
"""

import contextlib

import numpy as np
import ml_dtypes

BF16 = ml_dtypes.bfloat16

B, H, W, FIN = 8, 32, 32, 256
POS = H * W
FOUT, K, DK, DV, NH = 512, 3, 256, 256, 8
DKH, DVH = DK // NH, DV // NH
FOUT_CONV = FOUT - DV  # 256
N_CORES = 8

_PROG_CACHE = {}


def _build_program(variant="full"):
    """Build (and cache) the compiled Bass program. Same program for all 8
    cores (SPMD); per-core data arrives via the per-core input maps."""
    if ("nc", variant) in _PROG_CACHE:
        return _PROG_CACHE[("nc", variant)]

    import concourse.bass as bass
    import concourse.bacc as bacc
    import concourse.tile as tile
    from concourse import mybir

    BF = mybir.dt.bfloat16
    F32 = mybir.dt.float32
    EXP = mybir.ActivationFunctionType.Exp

    nc = bacc.Bacc("TRN2", target_bir_lowering=False, debug=False,
                   num_devices=N_CORES)

    # ---- DRAM parameters ----
    PADW = H + 2  # 34
    xpad_d = nc.dram_tensor("xpad", [FIN, PADW * PADW], BF, kind="ExternalInput")
    xtc_d = nc.dram_tensor("xtc", [FIN, POS], BF, kind="ExternalInput")
    wkqv_d = nc.dram_tensor("wkqv", [FIN, 2 * DK + DV], BF, kind="ExternalInput")
    wconv_d = nc.dram_tensor("wconv", [2, 128, K * K * FOUT_CONV], BF,
                             kind="ExternalInput")
    wproj_d = nc.dram_tensor("wproj", [DV, DV], BF, kind="ExternalInput")
    biases_d = nc.dram_tensor("biases", [128, 8], F32, kind="ExternalInput")
    krw_d = nc.dram_tensor("krw4", [128, 1024], BF, kind="ExternalInput")
    krh_d = nc.dram_tensor("krh4", [128, 1024], BF, kind="ExternalInput")
    delta_d = nc.dram_tensor("delta", [2 * W, POS], BF, kind="ExternalInput")
    out_d = nc.dram_tensor("out", [FOUT, POS], BF, kind="ExternalOutput")
    dbg_d = None
    if variant.startswith("debug"):
        dbg_d = nc.dram_tensor("dbg", [128, 4 * POS], BF, kind="ExternalOutput")
        dbg2_d = nc.dram_tensor("dbg2", [2, 128, POS], BF, kind="ExternalOutput")
        dbg3_d = nc.dram_tensor("dbg3", [2, 96, POS], BF, kind="ExternalOutput")
        dbg4_d = nc.dram_tensor("dbg4", [8, 128, POS], BF, kind="ExternalOutput")
        dbg5_d = nc.dram_tensor("dbg5", [8, DVH + 1, POS], BF, kind="ExternalOutput")

    xpad_in = xpad_d.ap()
    xtc = xtc_d.ap()
    wkqv = wkqv_d.ap()
    wconv = wconv_d.ap()
    wproj = wproj_d.ap()
    biases = biases_d.ap()
    krw = krw_d.ap()
    krh = krh_d.ap()
    delta = delta_d.ap()
    out = out_d.ap()

    with tile.TileContext(nc) as tc, contextlib.ExitStack() as ctx:
        consts = ctx.enter_context(tc.tile_pool(name="consts", bufs=1))
        xpads = ctx.enter_context(tc.tile_pool(name="xpads", bufs=1))
        kqpool = ctx.enter_context(tc.tile_pool(name="kqpool", bufs=1))
        vopool = ctx.enter_context(tc.tile_pool(name="vopool", bufs=1))
        attall = ctx.enter_context(tc.tile_pool(name="attall", bufs=1))
        megas = ctx.enter_context(tc.tile_pool(name="megas", bufs=1))
        stp = ctx.enter_context(tc.tile_pool(name="stp", bufs=1))
        cacc = ctx.enter_context(tc.tile_pool(name="cacc", bufs=1))
        work = ctx.enter_context(tc.tile_pool(name="work", bufs=4))
        small = ctx.enter_context(tc.tile_pool(name="small", bufs=4))
        outp = ctx.enter_context(tc.tile_pool(name="outp", bufs=2))
        # PSUM: 8 banks = ps_s 2x[128,1024]f32 (S chunks; startup kqv/rel)
        # + ps_at 1x (PV accumulator) + ps_w 1x (conv groups, prewarm, v,
        # tail proj/rcp-broadcast).
        ps_s = ctx.enter_context(tc.tile_pool(name="ps_s", bufs=2,
                                              space=bass.MemorySpace.PSUM))
        ps_at = ctx.enter_context(tc.tile_pool(name="ps_at", bufs=1,
                                               space=bass.MemorySpace.PSUM))
        ps_w = ctx.enter_context(tc.tile_pool(name="ps_w", bufs=1,
                                              space=bass.MemorySpace.PSUM))

        # ---- startup input DMAs across the 3 DMA queues (sync/scalar/
        # gpsimd). Effective per-queue bandwidth is only ~100-150 GB/s, so
        # the kqv-critical tensors are column-split and lead each queue;
        # bulk (wconv/xpad/wproj) trails. q-columns of wkqv go first: the
        # rel waves (longest prep chain) need only q + tables. ----
        # critical set first, balanced (~0.4-0.5MB each):
        #   sync:   xtc0, ball, delta0 | scalar: wkqv-q, krw | gpsimd:
        #   xtc1, krh, delta1 — then k/v cols, wconv, xpad, wproj trail
        xt_sb, wkqv_sb = [], []
        for f in range(2):
            xt_sb.append(xpads.tile([128, POS], BF, tag=f"xtsb{f}",
                                    name=f"xtsb{f}"))
            wkqv_sb.append(consts.tile([128, 2 * DK + DV], BF,
                                       tag=f"wkqv{f}", name=f"wkqv{f}"))
        for f in range(2):
            nc.scalar.dma_start(out=wkqv_sb[f][:, 256:512],
                                in_=wkqv[f * 128:(f + 1) * 128, 256:512])
        for f, eng in ((0, nc.sync), (1, nc.gpsimd)):
            for nh in range(2):
                eng.dma_start(out=xt_sb[f][:, nh * 512:(nh + 1) * 512],
                              in_=xtc[f * 128:(f + 1) * 128,
                                      nh * 512:(nh + 1) * 512])
        ball_sb = consts.tile([128, 8], F32, tag="ball")
        nc.sync.dma_start(out=ball_sb[:], in_=biases[:, :])
        bkq_sb = [ball_sb[:, cc:cc + 1] for cc in range(4)]
        bconv_sb = [ball_sb[:, 4 + co:5 + co] for co in range(2)]
        bproj_sb = [ball_sb[:, 6 + co:7 + co] for co in range(2)]
        krw_sb = consts.tile([128, 1024], BF, tag="krw")
        nc.scalar.dma_start(out=krw_sb[:], in_=krw[:, :])
        krh_sb = consts.tile([128, 1024], BF, tag="krh")
        nc.gpsimd.dma_start(out=krh_sb[:], in_=krh[:, :])

        # parity stationary tiles for S: rows 0:32 = k of the running head
        # (rewritten two heads ahead), rows 32:96 = one-hot deltas (static)
        st = []
        for i in range(2):
            t = stp.tile([96, POS], BF, tag=f"st{i}")
            eng = nc.sync if i == 0 else nc.gpsimd
            eng.dma_start(out=t[32:96, :], in_=delta[:, :])
            st.append(t)

        # k then v columns of wkqv (needed after q)
        for f in range(2):
            eng = nc.sync if f == 0 else nc.scalar
            eng.dma_start(out=wkqv_sb[f][:, 0:256],
                          in_=wkqv[f * 128:(f + 1) * 128, 0:256])
        for f in range(2):
            nc.scalar.dma_start(out=wkqv_sb[f][:, 512:768],
                                in_=wkqv[f * 128:(f + 1) * 128, 512:768])
        wproj_sb = []
        for f in range(2):
            t = consts.tile([128, DV], BF, tag=f"wproj{f}")
            nc.sync.dma_start(out=t[:], in_=wproj[f * 128:(f + 1) * 128, :])
            wproj_sb.append(t)

        wconv_sb = []
        for f in range(2):
            t = consts.tile([128, K * K * FOUT_CONV], BF, tag=f"wconv{f}")
            eng = nc.scalar if f == 0 else nc.gpsimd
            eng.dma_start(out=t[:], in_=wconv[f, :, :])
            wconv_sb.append(t)
        xpad = []
        for f in range(2):
            t = xpads.tile([128, PADW * PADW], BF, tag=f"xpad{f}")
            eng = nc.sync if f == 0 else nc.gpsimd
            eng.dma_start(out=t[:], in_=xpad_in[f * 128:(f + 1) * 128, :])
            xpad.append(t)

        # parity moving-operand tiles for S: rows 0:32 q, 32:64 rel_w,
        # 64:96 rel_h of the running head, assembled by 3 DMAs per head
        # (two heads ahead) from the kqv escape + rel staging tiles
        rh = []
        for i in range(2):
            rh.append(megas.tile([96, POS], BF, tag=f"rh{i}", name=f"rh{i}"))
        # rel staging: whole-wave psum escapes land here (4 heads each)
        stgw, stgh = [], []
        for w in range(2):
            stgw.append(megas.tile([128, POS], BF, tag=f"stgw{w}",
                                   name=f"stgw{w}"))
            stgh.append(megas.tile([128, POS], BF, tag=f"stgh{w}",
                                   name=f"stgh{w}"))

        ones_sb = consts.tile([1, 32], BF, tag="ones")
        nc.vector.memset(ones_sb[:], 1.0)
        ones128 = consts.tile([128, 1], BF, tag="ones128")
        nc.vector.memset(ones128[:], 1.0)

        # ---- PE pre-warm (memset-fed) bridges the input-DMA wait so the
        # HAM clock-gate is released before the first real matmul ----
        wu = consts.tile([128, 512], BF, tag="wu")
        nc.vector.memset(wu[:], 0.25)
        wups = ps_w.tile([128, POS], F32, tag="pw", name="wups")
        for i in range(6):
            nc.tensor.matmul(wups[:, 0:512], lhsT=wu[:, 0:128], rhs=wu[:],
                             start=True, stop=True)
        # preload the exp ACT table (~2.7us) long before the first real
        # exp (output goes back into the prewarm tile — values unused)
        nc.scalar.activation(out=wu[0:1, 0:32], in_=wu[0:1, 0:32], func=EXP)

        # ---- kqv sections (channel-major [co, pos]) + rel waves ----
        # Per q section: kqv matmuls -> escape -> 4-head-concurrent rel
        # matmuls reading q straight off the escape strips (krw4/krh4 are
        # strip-replicated host-side). rel_w for (wq, w') lands at
        # psum[32s + w', 32*wq + hq] and is escaped as ONE whole-tile DVE
        # copy with the (wq,hq)->(hq,wq) un-permute folded into the AP;
        # rel_h (contiguous) escapes as ONE ScalarE copy (ACT idle here).
        kq_sb = [None] * 4

        def kq_section(cc, pool=None, tag="ps"):
            pool = pool or ps_s
            ps = pool.tile([128, POS], F32, tag=tag, name=f"kqps{cc}")
            for f in range(2):
                for nh in range(2):
                    nc.tensor.matmul(
                        ps[:, nh * 512:(nh + 1) * 512],
                        lhsT=wkqv_sb[f][:, cc * 128:(cc + 1) * 128],
                        rhs=xt_sb[f][:, nh * 512:(nh + 1) * 512],
                        start=(f == 0), stop=(f == 1))
            t = kqpool.tile([128, POS], BF, tag=f"kq{cc}", name=f"kq{cc}")
            nc.vector.tensor_scalar_add(out=t[:], in0=ps[:], scalar1=bkq_sb[cc][:])
            kq_sb[cc] = t

        def relw_mm(w, pw, a, s):
            q3 = kq_sb[2 + w].rearrange("p (b a) -> p a b", a=W)
            nc.tensor.matmul(
                pw[32 * s:32 * s + 32, 32 * a:32 * a + 32],
                lhsT=krw_sb[32 * s:32 * s + 32, a * 32:(a + 1) * 32],
                rhs=q3[32 * s:32 * s + 32, a, :],
                start=True, stop=True, tile_position=(32 * s, 32 * s))

        def relh_mm(w, phh, a, s):
            nc.tensor.matmul(
                phh[32 * s:32 * s + 32, 32 * a:32 * a + 32],
                lhsT=krh_sb[32 * s:32 * s + 32, a * 32:(a + 1) * 32],
                rhs=kq_sb[2 + w][32 * s:32 * s + 32, a * 32:(a + 1) * 32],
                start=True, stop=True, tile_position=(32 * s, 32 * s))

        def relw_escape(w, pw, eng):
            # whole-wave escape, un-permute in the read AP
            eng.tensor_copy(
                out=stgw[w][:].rearrange("p (b a) -> p b a", a=W),
                in_=pw[:].rearrange("p (a b) -> p b a", a=W))

        def rel_wave(w, pool=None, tag="ps"):
            pool = pool or ps_s
            pw = pool.tile([128, POS], F32, tag=tag, name=f"relw{w}")
            for a in range(W):
                for s in range(4):
                    relw_mm(w, pw, a, s)
            relw_escape(w, pw, nc.vector)
            phh = pool.tile([128, POS], F32, tag=tag, name=f"relh{w}")
            for a in range(W):
                for s in range(4):
                    relh_mm(w, phh, a, s)
            nc.scalar.copy(out=stgh[w][:], in_=phh[:])

        def assemble(h, qeng, releng, parity):
            # build the rh parity tile for head h (q + rel rows)
            w, s = h // 4, h % 4
            qeng.dma_start(out=rh[parity][0:32, :],
                           in_=kq_sb[2 + w][32 * s:32 * s + 32, :])
            releng.dma_start(out=rh[parity][32:64, :],
                             in_=stgw[w][32 * s:32 * s + 32, :])
            releng.dma_start(out=rh[parity][64:96, :],
                             in_=stgh[w][32 * s:32 * s + 32, :])

        kq_section(2)
        # HAM-bridge matmuls: keyed on the q escape so they fill the
        # otherwise-idle wait for the rel tables to land, keeping the PE
        # activity monitor from re-throttling to half clock before the
        # (issue-bound) rel waves
        brg = ps_w.tile([128, POS], F32, tag="pw", name="brg")
        for i in range(6):
            nc.tensor.matmul(brg[:, 0:512], lhsT=wu[:, 0:128],
                             rhs=kq_sb[2][:, 0:512], start=True, stop=True)
        rel_wave(0)
        kq_section(0)
        for h in range(2):  # k rows of h0/h1 prime the st parity tiles
            nc.sync.dma_start(out=st[h][0:32, :],
                              in_=kq_sb[0][32 * h:32 * h + 32, :])
        kq_section(3)
        rel_wave(1)
        kq_section(1)
        assemble(0, nc.sync, nc.sync, parity=0)
        assemble(1, nc.gpsimd, nc.gpsimd, parity=1)

        # ---- v: position-major [pos, dv] -> vomega with ones interleave ----
        vomega = vopool.tile([128, 8 * NH * (DVH + 1)], BF, tag="vomega")
        vom4 = vomega.rearrange("p (k h d) -> p k h d", k=8, d=DVH + 1)
        nc.vector.memset(vom4[:, :, :, DVH:DVH + 1], 1.0)
        for half in range(2):
            ps = ps_w.tile([128, POS], F32, tag="pw", name=f"vps{half}")
            for q in range(4):
                kc = half * 4 + q
                for f in range(2):
                    nc.tensor.matmul(
                        ps[:, q * 256:(q + 1) * 256],
                        lhsT=xt_sb[f][:, kc * 128:(kc + 1) * 128],
                        rhs=wkqv_sb[f][:, 2 * DK:2 * DK + DV],
                        start=(f == 0), stop=(f == 1))
            nc.vector.tensor_copy(
                out=vom4[:, half * 4:(half + 1) * 4, :, 0:DVH],
                in_=ps.rearrange("p (k h d) -> p k h d", k=4, d=DVH))

        att_all = []
        for f in range(2):
            t = attall.tile([128, POS], BF, tag=f"att{f}", name=f"att{f}")
            att_all.append(t)
        # conv fp32 SBUF accumulators (per co half; each (nh, f) group's
        # 9-tap psum result is folded in with a DVE copy/add)
        conv_acc = []
        for co in range(2):
            t = cacc.tile([128, POS], F32, tag=f"cacc{co}")
            conv_acc.append(t)

        def xwin(f, dy, dx, h0, hn):
            # [128, hn, 32] window of the padded image
            t3 = xpad[f].rearrange("p (a b) -> p a b", a=PADW)
            return t3[:, h0 + dy:h0 + dy + hn, dx:dx + W]

        def conv_escape(co):
            if variant == "debug_noconv":
                return
            ot = outp.tile([128, POS], BF, tag="out", name=f"cot{co}")
            nc.vector.tensor_scalar_add(out=ot[:], in0=conv_acc[co][:],
                                        scalar1=bconv_sb[co][:])
            # co=1 lands at the tail: its out-DMA rides gpsimd, parallel
            # with the proj out-DMAs on sync
            eng = nc.sync if co == 0 else nc.gpsimd
            eng.dma_start(out=out[co * 128:(co + 1) * 128, :], in_=ot[:])

        # ---- per-head attention, one global pipelined chunk stream.
        # Head order rotates section 1 so the LAST head (4) owns att_all
        # partitions 0:32 — its normalize then writes att_all directly on
        # DVE (no partition-move DMA on the critical tail).
        # PE filler rides the chunk slots: iterations 0/1 carry the
        # wave-1 rel matmuls (through the ps_w tile, so wave 1 never
        # blocks the ps_s rotation ahead of the first exp); iterations
        # 2-7 carry the conv taps, 12 per iteration. ----
        ORDER = [0, 1, 2, 3, 5, 6, 7, 4]
        FILL = [[[] for _ in range(8)] for _ in range(8)]
        END = [[] for _ in range(8)]
        holder = {}

        def conv_tap(g, tp):
            if variant == "debug_noconv":
                return
            co, nh, f = g // 4, (g // 2) % 2, g % 2
            if tp == 0:
                holder[f"cps{g}"] = ps_w.tile([128, POS], F32, tag="pw",
                                              name=f"cps{g}")
            cps = holder[f"cps{g}"]
            dy, dx = tp // 3, tp % 3
            o0 = tp * FOUT_CONV + co * 128
            nc.tensor.matmul(
                cps[:, nh * 512:(nh + 1) * 512],
                lhsT=wconv_sb[f][:, o0:o0 + 128],
                rhs=xwin(f, dy, dx, nh * 16, 16),
                start=(tp == 0), stop=(tp == 8))
            if tp == 8:
                acc = conv_acc[co][:, nh * 512:(nh + 1) * 512]
                src = cps[:, nh * 512:(nh + 1) * 512]
                if f == 0:
                    nc.vector.tensor_copy(out=acc, in_=src)
                else:
                    nc.vector.tensor_add(acc, acc, src)

        # conv group `it` rides iteration `it`: 9 taps over its 8 slots
        for it in range(8):
            for kc in range(8):
                for tp in range(kc * 9 // 8, (kc + 1) * 9 // 8):
                    FILL[it][kc].append(lambda g=it, tp=tp: conv_tap(g, tp))

        # per-iteration state for the flat pipeline
        at_t = [None] * 8
        psb_t = [[None] * 8 for _ in range(8)]

        def s_step(it, kc):
            h = ORDER[it]
            sps = ps_s.tile([128, POS], F32, tag="ps", name=f"sps{h}_{kc}")
            for nh2 in range(2):
                nc.tensor.matmul(
                    sps[:, nh2 * 512:(nh2 + 1) * 512],
                    lhsT=st[it % 2][0:96, kc * 128:(kc + 1) * 128],
                    rhs=rh[it % 2][0:96, nh2 * 512:(nh2 + 1) * 512],
                    start=True, stop=True)
            psb = work.tile([128, POS], BF, tag="pexp", name=f"psb{h}_{kc}")
            nc.scalar.activation(out=psb[:], in_=sps[:], func=EXP)
            psb_t[it][kc] = psb
            if dbg_d is not None and h == 0:
                nc.sync.dma_start(out=dbg4_d.ap()[kc, :, :], in_=psb[:])

        def pv_step(it, kc):
            h = ORDER[it]
            if kc == 0:
                at_t[it] = ps_at.tile([128, POS], F32, tag="at",
                                      name=f"at{h}")
            nc_k = kc * NH * (DVH + 1) + h * (DVH + 1)
            for nh2 in range(2):
                nc.tensor.matmul(
                    at_t[it][0:DVH + 1, nh2 * 512:(nh2 + 1) * 512],
                    lhsT=vomega[:, nc_k:nc_k + DVH + 1],
                    rhs=psb_t[it][kc][:, nh2 * 512:(nh2 + 1) * 512],
                    start=(kc == 0), stop=(kc == 7))


        def end_head(it):
            h = ORDER[it]
            at = at_t[it]
            for thunk in END[it]:
                thunk()
            # k/q/rel rows of the head two iterations out, into this
            # iteration's parity tiles (overlap the next iteration; Tile
            # orders them after the last S read)
            if it + 2 < 8:
                h2 = ORDER[it + 2]
                nc.sync.dma_start(
                    out=st[it % 2][0:32, :],
                    in_=kq_sb[0 if h2 < 4 else 1][32 * (h2 % 4):
                                                  32 * (h2 % 4) + 32, :])
                assemble(h2, nc.sync, nc.gpsimd, parity=it % 2)

            # psum-escape copy (frees rows 0:33 for the next head), then
            # normalize: attn_h = (P^T V)[0:32] / sumexp (row 32)
            sec = h // 4
            g = (h % 4) * 32
            cmb = small.tile([DVH + 1, POS], BF, tag="cmb", name=f"cmb{h}")
            nc.vector.tensor_copy(out=cmb[:], in_=at[0:DVH + 1, :])
            if dbg_d is not None and it < 7:
                nc.sync.dma_start(out=dbg5_d.ap()[h, :, :], in_=cmb[:])
            gshape = [32, 32] if it == 7 else [128, 8]
            s8 = small.tile(gshape, BF, tag="s8", name=f"s8{h}")
            nc.gpsimd.dma_start(out=s8[:], in_=cmb[DVH:DVH + 1, :])
            rcp8 = small.tile(gshape, BF, tag="rcp8", name=f"rcp8{h}")
            with nc.allow_low_precision(reason="1/sumexp in bf16 is "
                                        "within the softmax budget"):
                nc.vector.reciprocal(out=rcp8[:], in_=s8[:])
            rcpf = small.tile([1, POS], BF, tag="rcpf", name=f"rcpf{h}")
            nc.sync.dma_start(out=rcpf[:], in_=rcp8[:])
            if it == 7:
                # tail fast path (head 4 = att_all[1] rows 0:32):
                # broadcast 1/sumexp via a K=1 PE matmul and write att_all
                # directly on DVE — no partition-move DMA on the tail
                rps = ps_w.tile([128, POS], F32, tag="pw", name="rcppe")
                for nh2 in range(2):
                    nc.tensor.matmul(
                        rps[0:32, nh2 * 512:(nh2 + 1) * 512],
                        lhsT=ones_sb[:, :],
                        rhs=rcpf[:, nh2 * 512:(nh2 + 1) * 512],
                        start=True, stop=True)
                nc.vector.tensor_mul(att_all[sec][g:g + 32, :],
                                     cmb[0:DVH, :], rps[0:32, :])
                return
            an = small.tile([32, POS], BF, tag="an", name=f"an{h}")
            rcpb = small.tile([32, POS], BF, tag="rcpb", name=f"rcpb{h}")
            nc.gpsimd.partition_broadcast(rcpb[:], rcpf[:])
            nc.vector.tensor_mul(an[:], cmb[0:DVH, :], rcpb[:])
            nc.gpsimd.dma_start(out=att_all[sec][g:g + 32, :], in_=an[:])

        # nested per-head chunk pipeline (PV lag 1)
        for it in range(8):
            s_step(it, 0)
            for thunk in FILL[it][0]:
                thunk()
            s_step(it, 1)
            for thunk in FILL[it][1]:
                thunk()
            pv_step(it, 0)
            for kc in range(2, 8):
                s_step(it, kc)
                for thunk in FILL[it][kc]:
                    thunk()
                pv_step(it, kc - 1)
            pv_step(it, 7)
            end_head(it)
            if it == 4:
                conv_escape(0)
        conv_escape(1)
        if dbg_d is not None:
            for w in range(2):
                nc.sync.dma_start(out=dbg_d.ap()[:, (2 * w) * POS:
                                                 (2 * w + 1) * POS],
                                  in_=stgw[w][:])
                nc.sync.dma_start(out=dbg_d.ap()[:, (2 * w + 1) * POS:
                                                 (2 * w + 2) * POS],
                                  in_=stgh[w][:])
            for f in range(2):
                nc.sync.dma_start(out=dbg2_d.ap()[f, :, :], in_=att_all[f][:])
                nc.sync.dma_start(out=dbg3_d.ap()[f, :, :], in_=st[f][:])

        # ---- tail: output projection accumulates in psum in three parts:
        # f=0 (ready since head 3), then f=1 rows 32:128 (heads 5-7, done
        # by iteration 6), and LAST a K=32 matmul against head 4's rows —
        # the only matmul gated on the final normalize. Escapes split
        # across DVE and the (idle) ScalarE ----
        pps = [ps_s.tile([128, POS], F32, tag="ps", name="pps0"),
               ps_s.tile([128, POS], F32, tag="ps", name="pps1")]
        for f in range(2):
            for co in range(2):
                for nh2 in range(2):
                    nc.tensor.matmul(
                        pps[co][:, nh2 * 512:(nh2 + 1) * 512],
                        lhsT=wproj_sb[f][:, co * 128:(co + 1) * 128],
                        rhs=att_all[f][:, nh2 * 512:(nh2 + 1) * 512],
                        start=(f == 0), stop=(f == 1))
        for co in range(2):
            ot = outp.tile([128, POS], BF, tag="out", name=f"pot{co}")
            if co == 0:
                nc.vector.tensor_scalar_add(out=ot[:], in0=pps[co][:],
                                            scalar1=bproj_sb[co][:])
            else:
                nc.scalar.activation(out=ot[:], in_=pps[co][:],
                                     func=mybir.ActivationFunctionType.Identity,
                                     bias=bproj_sb[co][:], scale=1.0)
            nc.sync.dma_start(
                out=out[FOUT_CONV + co * 128:FOUT_CONV + (co + 1) * 128, :],
                in_=ot[:])

    nc.compile()
    _PROG_CACHE[("nc", variant)] = nc
    return nc


def _host_prep(x, w_kqv, b_kqv, w_proj, b_proj, w_conv, b_conv,
               key_rel_w, key_rel_h):
    """Layout-only host prep -> per-core input maps."""
    x = np.asarray(x, np.float32)
    w_kqv = np.asarray(w_kqv, np.float32)
    b_kqv = np.asarray(b_kqv, np.float32)
    w_proj = np.asarray(w_proj, np.float32)
    b_proj = np.asarray(b_proj, np.float32)
    w_conv = np.asarray(w_conv, np.float32)
    b_conv = np.asarray(b_conv, np.float32)
    key_rel_w = np.asarray(key_rel_w, np.float32)
    key_rel_h = np.asarray(key_rel_h, np.float32)

    scale = np.float32(DKH ** -0.5)
    wkqv = w_kqv.copy()
    wkqv[:, DK:2 * DK] *= scale           # fold q scaling into the weights
    bkq = b_kqv[:2 * DK].copy()
    bkq[DK:] *= scale
    # fold the v bias through the projection: attn = (attn0 + bv) Wp + bp
    bproj_eff = b_proj + b_kqv[2 * DK:] @ w_proj
    # combined per-partition bias tile [128, 8]:
    # cols 0-3 = b_kq 128-chunks, 4-5 = b_conv chunks, 6-7 = b_proj chunks
    ball = np.stack([bkq[0:128], bkq[128:256], bkq[256:384], bkq[384:512],
                     b_conv[0:128], b_conv[128:256],
                     bproj_eff[0:128], bproj_eff[128:256]], axis=1)

    # window-expanded relative tables, replicated to all 4 partition groups:
    #   krw4[32r + d, wq*32 + w'] = key_rel_w[w' - wq + 31, d]
    idx = (np.arange(W)[None, :] - np.arange(W)[:, None] + (W - 1))  # [wq, w']
    krw = key_rel_w[idx]                   # [wq, w', 32]
    krw4 = np.tile(krw.transpose(2, 0, 1).reshape(DKH, W * W), (4, 1))
    krh = key_rel_h[idx]
    krh4 = np.tile(krh.transpose(2, 0, 1).reshape(DKH, H * H), (4, 1))

    # one-hot offset deltas: rows 0-31 wk one-hots, rows 32-63 hk one-hots
    kpos = np.arange(POS)
    deltas = np.zeros((2 * W, POS), np.float32)
    deltas[kpos % W, kpos] = 1.0
    deltas[W + kpos // W, kpos] = 1.0

    # conv weights repacked so each 128-channel chunk's 9 taps are one
    # contiguous per-partition run: wconv[f][p, tp*256 + o]
    wc = w_conv.reshape(K * K, 2, 128, FOUT_CONV)          # [tap, f, p, o]
    wc = np.ascontiguousarray(wc.transpose(1, 2, 0, 3)).reshape(
        2, 128, K * K * FOUT_CONV)

    shared = {
        "wkqv": wkqv.astype(BF16),
        "wconv": wc.astype(BF16),
        "wproj": w_proj.astype(BF16),
        "biases": ball.astype(np.float32),
        "krw4": krw4.astype(BF16),
        "krh4": krh4.astype(BF16),
        "delta": deltas.astype(BF16),
    }
    PADW = H + 2
    in_maps = []
    for b in range(N_CORES):
        m = dict(shared)
        xt = np.ascontiguousarray(x[b].reshape(POS, FIN).T)   # [FIN, POS]
        xp = np.zeros((FIN, PADW, PADW), np.float32)
        xp[:, 1:H + 1, 1:W + 1] = xt.reshape(FIN, H, W)
        m["xpad"] = xp.reshape(FIN, PADW * PADW).astype(BF16)
        m["xtc"] = xt.astype(BF16)
        in_maps.append(m)
    return in_maps


def kernel(x, w_kqv, b_kqv, w_proj, b_proj, w_conv, b_conv,
           key_rel_w, key_rel_h):
    from concourse.bass_utils import run_bass_kernel_spmd

    nc = _build_program()
    in_maps = _host_prep(x, w_kqv, b_kqv, w_proj, b_proj, w_conv, b_conv,
                         key_rel_w, key_rel_h)
    if not _PROG_CACHE.get("warm"):
        # first execution in a process runs ~15-20% slower (cold NEFF/DMA/
        # clock state); one throwaway execution warms the device
        run_bass_kernel_spmd(nc, in_maps, core_ids=list(range(N_CORES)))
        _PROG_CACHE["warm"] = True
    res = run_bass_kernel_spmd(nc, in_maps, core_ids=list(range(N_CORES)))
    out = np.empty((B, H, W, FOUT), np.float32)
    for b in range(N_CORES):
        out[b] = res.results[b]["out"].T.reshape(H, W, FOUT)
    return out
